# revision 11
# baseline (speedup 1.0000x reference)
"""Trainium2 Bass kernel for DefaultGIN (gnn_message_passing), 8-core SPMD.

Math: with a 1-row embedding table and x==0, every node's input feature is
emb[0], so conv1's output depends only on each node's in-degree d:
    h1[i] = relu(mlp1((1+deg_i) * e))          (table F[d], d < C classes)
Conv2's pre-activation collapses to a per-node class-count matmul:
    pre2a[i] = (h1[i] + sum_j h1[j]) @ w2a + b2a
             = CNT[i] @ G'            where G'[d] = F[d] @ w2a, plus a bias row,
    CNT[i,d] = #{in-neighbors j: deg_j = d} + 1[deg_i = d]
(the own term enters as a host-added self-loop edge), and the trailing
linear layers commute with mean pooling:
    out = (sum_g relu(pre2a) @ (w2b @ wf)) / count_g + (b2b @ wf + bf).

Device does all float math: tables, per-edge scatter-accumulation of CNT
(one-hot compare + PE matmul per 128-edge chunk; counts are exact in bf16),
pre2a via a hi/lo bf16 split of G' (near-fp32 precision at bf16 speed),
relu, pooling, tail matmuls, cross-core AllReduce, and the mean division.
Host only shards and labels edges (sort by dst, degree-class ints) and
builds iota/identity constants.
"""

import math

import numpy as np
import ml_dtypes

import concourse.bass as bass
import concourse.bacc as bacc
import concourse.mybir as mybir
import concourse.tile as tile
from concourse import bass_utils

P = 128
NCORE = 8
D = 256
D2 = D + 2          # pooled row: 256 features + count + zero pad (even for PE)
DOUT = 32
GRAPHS = 512
F32 = mybir.dt.float32
BF16 = mybir.dt.bfloat16
SENT = 9999.0  # one-hot sentinel; != any iota value in bf16/f32


def _build_program(C, NT, ct, W):
    """Build the 8-core SPMD Bass program. Shapes only depend on C/NT/ct/W."""
    CH = NT * ct
    NW = W // P  # pooling window 128-blocks
    nc = bacc.Bacc("TRN2", target_bir_lowering=False)

    def din(name, shape, dt):
        return nc.dram_tensor(name, shape, dt, kind="ExternalInput")

    # per-core edge/node labels
    dstrel_t = din("dstrel", [P, CH], BF16)
    clssrc_t = din("clssrc", [P, CH], BF16)
    batchrel_t = din("batchrel", [P, NT], BF16)
    pmat_t = din("pmat", [W, GRAPHS], F32)
    # constants
    iota_s_t = din("iota_s", [P, ct * P], BF16)
    iota_c_t = din("iota_c", [P, ct * C], BF16)
    iota_g_t = din("iota_g", [P, W], BF16)
    ident_f_t = din("ident_f", [P, P], F32)
    ones_row_t = din("ones_row", [1, P], F32)
    dplus1_t = din("dplus1", [P, C], F32)
    # weights
    emb_t = din("emb", [1, D], F32)
    w1a_t = din("w1a", [D, D], F32)
    b1a_t = din("b1a", [D], F32)
    w1b_t = din("w1b", [D, D], F32)
    b1b_t = din("b1b", [D], F32)
    w2a_t = din("w2a", [D, D], F32)
    b2a_t = din("b2a", [D], F32)
    w2b_t = din("w2b", [D, D], F32)
    b2b_t = din("b2b", [D], F32)
    wf_t = din("wf", [D, DOUT], F32)
    bf_t = din("bf", [DOUT], F32)

    out_t = nc.dram_tensor("out", [GRAPHS, DOUT], F32, kind="ExternalOutput")

    pg_t = nc.dram_tensor("pg", [GRAPHS, DOUT + 2], F32)
    pr_t = nc.dram_tensor("pr", [GRAPHS, DOUT + 2], F32, addr_space="Shared")

    IS_EQ = mybir.AluOpType.is_equal
    MULT = mybir.AluOpType.mult
    ADD = mybir.AluOpType.add
    SUB = mybir.AluOpType.subtract
    RELU = mybir.ActivationFunctionType.Relu

    with tile.TileContext(nc) as tc:
        with (
            tc.tile_pool(name="const", bufs=1) as cp,
            tc.tile_pool(name="work", bufs=4) as wp,
            tc.tile_pool(name="psum", bufs=5, space="PSUM") as pp,
            tc.tile_pool(name="pwin", bufs=1, space="PSUM") as pwp,
        ):
            # ---- load constants & weights ----
            def load(pool, t):
                tl = pool.tile(list(t.shape), t.dtype, tag=t.name)
                nc.sync.dma_start(out=tl[:], in_=t[:])
                return tl

            dstrel = load(cp, dstrel_t)
            clssrc = load(cp, clssrc_t)
            batchrel = load(cp, batchrel_t)
            iota_s = load(cp, iota_s_t)
            iota_c = load(cp, iota_c_t)
            iota_g = load(cp, iota_g_t)
            ident_f = load(cp, ident_f_t)
            ones_row = load(cp, ones_row_t)
            dplus1 = load(cp, dplus1_t)

            def load_chunks(t, n, cols):
                out = []
                for kc in range(n):
                    tl = cp.tile([P, cols], F32, tag=f"{t.name}_ch{kc}")
                    nc.sync.dma_start(out=tl[:], in_=t[kc * P:(kc + 1) * P, :])
                    out.append(tl)
                return out

            w1a_sb = load_chunks(w1a_t, 2, D)
            w1b_sb = load_chunks(w1b_t, 2, D)
            w2a_sb = load_chunks(w2a_t, 2, D)
            w2b_sb = load_chunks(w2b_t, 2, D)
            wf_sb = load_chunks(wf_t, 2, DOUT)
            pmat_sb = load_chunks(pmat_t, NW, GRAPHS)

            def load_row(t, n):
                tl = cp.tile([1, n], F32, tag=f"{t.name}_row")
                nc.sync.dma_start(out=tl[:], in_=t[None, :])
                return tl

            b1a_row = load_row(b1a_t, D)
            b1b_row = load_row(b1b_t, D)
            b2a_row = load_row(b2a_t, D)
            bf_row = load_row(bf_t, DOUT)
            b2b_col = []
            for kc in range(2):
                tl = cp.tile([P, 1], F32, tag=f"b2bc{kc}")
                nc.sync.dma_start(out=tl[:], in_=b2b_t[kc * P:(kc + 1) * P, None])
                b2b_col.append(tl)
            e_chunk = []
            for kc in range(2):
                tl = cp.tile([P, 1], F32, tag=f"ech{kc}")
                nc.sync.dma_start(out=tl[:], in_=emb_t[0, kc * P:(kc + 1) * P, None])
                e_chunk.append(tl)

            # ---- P0: degree-class tables F, G' and folded tail weights ----
            def mlp_layer(lhsT_chunks, w_chunks, b_row, relu_out_tile):
                ps = pp.tile([P, D2], F32, space="PSUM", tag="ps")
                for kc in range(2):
                    nc.tensor.matmul(out=ps[0:C, 0:D], lhsT=lhsT_chunks[kc][:],
                                     rhs=w_chunks[kc][:], start=(kc == 0), stop=False)
                nc.tensor.matmul(out=ps[0:C, 0:D], lhsT=ones_row[:, 0:C], rhs=b_row[:],
                                 start=False, stop=True)
                nc.scalar.activation(out=relu_out_tile[:], in_=ps[0:C, 0:D], func=RELU)

            def transpose_cols(src_tile, np_, tags):
                # src [np_, 256] f32 -> two sbuf tiles [128, np_] (k-chunks)
                out = []
                for kc in range(2):
                    ps = pp.tile([P, D2], F32, space="PSUM", tag="ps")
                    nc.tensor.transpose(out=ps[0:P, 0:np_],
                                        in_=src_tile[:, kc * P:(kc + 1) * P],
                                        identity=ident_f[0:np_, 0:np_])
                    tl = wp.tile([P, np_], F32, tag=f"{tags}{kc}")
                    nc.vector.tensor_copy(out=tl[:], in_=ps[0:P, 0:np_])
                    out.append(tl)
                return out

            z0T = []
            for kc in range(2):
                tl = wp.tile([P, C], F32, tag=f"z0T{kc}")
                nc.vector.tensor_tensor(out=tl[:], in0=e_chunk[kc][:].to_broadcast([P, C]),
                                        in1=dplus1[:], op=MULT)
                z0T.append(tl)
            h1 = cp.tile([C, D], F32, tag="h1")
            mlp_layer(z0T, w1a_sb, b1a_row, h1)
            h1T = transpose_cols(h1, C, "h1T")
            fv = cp.tile([C, D], F32, tag="fv")
            mlp_layer(h1T, w1b_sb, b1b_row, fv)
            fvT = transpose_cols(fv, C, "fvT")
            gps = pp.tile([P, D2], F32, space="PSUM", tag="ps")
            for kc in range(2):
                nc.tensor.matmul(out=gps[0:C, 0:D], lhsT=fvT[kc][:], rhs=w2a_sb[kc][:],
                                 start=(kc == 0), stop=(kc == 1))
            # G'' staging (f32): rows 0..C-1 = F@w2a, row C = b2a; col 256 marks
            # the bias row (always-1 count -> relu(1)=1 feeds pooled counts),
            # col 257 = 0.
            gp_f = cp.tile([C + 1, D2], F32, tag="gp_f")
            nc.vector.tensor_copy(out=gp_f[0:C, 0:D], in_=gps[0:C, 0:D])
            nc.vector.tensor_copy(out=gp_f[C:C + 1, 0:D], in_=b2a_row[:])
            nc.gpsimd.memset(gp_f[0:C + 1, D:D2], 0.0)
            nc.vector.tensor_copy(out=gp_f[C:C + 1, D:D + 1], in_=ones_row[:, 0:1])
            # hi/lo bf16 split: gp_hi + gp_lo ~= gp_f to ~16 mantissa bits
            gp_hi = cp.tile([C + 1, D2], BF16, tag="gp_hi")
            nc.vector.tensor_copy(out=gp_hi[:], in_=gp_f[:])
            gp_lo_f = cp.tile([C + 1, D2], F32, tag="gp_lo_f")
            nc.vector.tensor_tensor(out=gp_lo_f[:], in0=gp_f[:], in1=gp_hi[:], op=SUB)
            gp_lo = cp.tile([C + 1, D2], BF16, tag="gp_lo")
            nc.vector.tensor_copy(out=gp_lo[:], in_=gp_lo_f[:])

            # W2 = w2b @ wf  (needs w2b blocks transposed), c = b2b @ wf + bf
            w2sb = []
            for ic in range(2):
                ps = pp.tile([P, D2], F32, space="PSUM", tag="ps")
                for kc in range(2):
                    tps = pp.tile([P, D2], F32, space="PSUM", tag="ps")
                    nc.tensor.transpose(out=tps[0:P, 0:P],
                                        in_=w2b_sb[ic][:, kc * P:(kc + 1) * P],
                                        identity=ident_f[:])
                    tsb = wp.tile([P, P], F32, tag="w2bT")
                    nc.vector.tensor_copy(out=tsb[:], in_=tps[0:P, 0:P])
                    nc.tensor.matmul(out=ps[0:P, 0:DOUT], lhsT=tsb[:], rhs=wf_sb[kc][:],
                                     start=(kc == 0), stop=(kc == 1))
                tl = cp.tile([P, DOUT], F32, tag=f"w2sb{ic}")
                nc.vector.tensor_copy(out=tl[:], in_=ps[0:P, 0:DOUT])
                w2sb.append(tl)
            cps = pp.tile([P, D2], F32, space="PSUM", tag="ps")
            for kc in range(2):
                nc.tensor.matmul(out=cps[0:1, 0:DOUT], lhsT=b2b_col[kc][:],
                                 rhs=wf_sb[kc][:], start=(kc == 0), stop=False)
            nc.tensor.matmul(out=cps[0:1, 0:DOUT], lhsT=ones_row[:, 0:1], rhs=bf_row[:],
                             start=False, stop=True)
            c_row = cp.tile([1, DOUT], F32, tag="c_row")
            nc.vector.tensor_copy(out=c_row[:], in_=cps[0:1, 0:DOUT])
            cbc_ps = pp.tile([P, D2], F32, space="PSUM", tag="ps")
            nc.tensor.matmul(out=cbc_ps[0:P, 0:DOUT], lhsT=ones_row[:], rhs=c_row[:],
                             start=True, stop=True)
            c_bc = cp.tile([P, DOUT], F32, tag="c_bc")
            nc.vector.tensor_copy(out=c_bc[:], in_=cbc_ps[0:P, 0:DOUT])

            # ---- P1: per-tile edge aggregation, pre2a, relu, pooling ----
            pw_ps = [pwp.tile([P, D2], F32, space="PSUM", tag=f"pw{h}",
                              name=f"pw{h}")
                     for h in range(NW)]
            for t in range(NT):
                s_all = wp.tile([P, ct * P], BF16, tag="s_all")
                nc.vector.tensor_tensor(
                    out=s_all[:],
                    in0=dstrel[:, t * ct:(t + 1) * ct].to_broadcast([P, ct, P]),
                    in1=iota_s[:], op=IS_EQ)
                ohc = wp.tile([P, ct * C], BF16, tag="ohc")
                nc.vector.tensor_tensor(
                    out=ohc[:],
                    in0=clssrc[:, t * ct:(t + 1) * ct].to_broadcast([P, ct, C]),
                    in1=iota_c[:], op=IS_EQ)

                ctps = pp.tile([P, D2], F32, space="PSUM", tag="ps")
                for c in range(ct):
                    nc.tensor.matmul(out=ctps[0:C, 0:P], lhsT=ohc[:, c * C:(c + 1) * C],
                                     rhs=s_all[:, c * P:(c + 1) * P],
                                     start=(c == 0), stop=(c == ct - 1))

                ctsb = wp.tile([C + 1, P], BF16, tag="ctsb")
                nc.vector.tensor_copy(out=ctsb[0:C, :], in_=ctps[0:C, 0:P])
                nc.vector.tensor_copy(out=ctsb[C:C + 1, :], in_=ones_row[:])

                preps = pp.tile([P, D2], F32, space="PSUM", tag="ps")
                nc.tensor.matmul(out=preps[:], lhsT=ctsb[:], rhs=gp_hi[:],
                                 start=True, stop=False)
                nc.tensor.matmul(out=preps[:], lhsT=ctsb[:], rhs=gp_lo[:],
                                 start=False, stop=True)
                msb = wp.tile([P, D2], BF16, tag="msb")
                nc.scalar.activation(out=msb[:], in_=preps[:], func=RELU)

                ohg = wp.tile([P, W], BF16, tag="ohg")
                nc.vector.tensor_tensor(
                    out=ohg[:], in0=batchrel[:, t:t + 1].to_broadcast([P, W]),
                    in1=iota_g[:], op=IS_EQ)
                for h in range(NW):
                    nc.tensor.matmul(out=pw_ps[h][:], lhsT=ohg[:, h * P:(h + 1) * P],
                                     rhs=msb[:], start=(t == 0), stop=(t == NT - 1),
                                     skip_group_check=True)

            # ---- P2: local tail matmul, placement, small AllReduce, mean ----
            pw_sb = []
            for h in range(NW):
                tl = wp.tile([P, D2], F32, tag=f"pwsb{h}")
                nc.vector.tensor_copy(out=tl[:], in_=pw_ps[h][:])
                pw_sb.append(tl)
            # per window block h: pre_h = pooled_h @ W2 (divide later), + counts
            pre_h_sb = []
            for h in range(NW):
                ops = pp.tile([P, D2], F32, space="PSUM", tag="ps")
                for fc in range(2):
                    tps = pp.tile([P, D2], F32, space="PSUM", tag="ps")
                    nc.tensor.transpose(out=tps[0:P, 0:P],
                                        in_=pw_sb[h][:, fc * P:(fc + 1) * P],
                                        identity=ident_f[:])
                    pwT = wp.tile([P, P], F32, tag="pwT")
                    nc.vector.tensor_copy(out=pwT[:], in_=tps[0:P, 0:P])
                    nc.tensor.matmul(out=ops[0:P, 0:DOUT], lhsT=pwT[:],
                                     rhs=w2sb[fc][:], start=(fc == 0), stop=(fc == 1))
                tl = wp.tile([P, DOUT + 2], F32, tag=f"preh{h}")
                nc.vector.tensor_copy(out=tl[:, 0:DOUT], in_=ops[0:P, 0:DOUT])
                nc.vector.tensor_copy(out=tl[:, DOUT:DOUT + 1],
                                      in_=pw_sb[h][:, D:D + 1])
                nc.vector.tensor_copy(out=tl[:, DOUT + 1:DOUT + 2],
                                      in_=pw_sb[h][:, D + 1:D + 2])
                pre_h_sb.append(tl)
            write_dmas = []
            for gt in range(GRAPHS // P):
                ps = pp.tile([P, D2], F32, space="PSUM", tag="ps")
                for h in range(NW):
                    nc.tensor.matmul(out=ps[0:P, 0:DOUT + 2],
                                     lhsT=pmat_sb[h][:, gt * P:(gt + 1) * P],
                                     rhs=pre_h_sb[h][:], start=(h == 0),
                                     stop=(h == NW - 1))
                tl = wp.tile([P, DOUT + 2], F32, tag="plsb")
                nc.vector.tensor_copy(out=tl[:], in_=ps[0:P, 0:DOUT + 2])
                dma = nc.sync.dma_start(out=pg_t[gt * P:(gt + 1) * P, :], in_=tl[:])
                write_dmas.append(dma)

            cc = nc.gpsimd.collective_compute(
                "AllReduce", ADD, replica_groups=[list(range(NCORE))],
                ins=[pg_t[:]], outs=[pr_t[:]])
            for dma in write_dmas:
                tile.add_dep_helper(cc.ins, dma.ins, reason="allreduce after writes")

            for gt in range(GRAPHS // P):
                prt = wp.tile([P, DOUT + 2], F32, tag="prt")
                dma = nc.sync.dma_start(out=prt[:], in_=pr_t[gt * P:(gt + 1) * P, :])
                tile.add_dep_helper(dma.ins, cc.ins, reason="read after allreduce")
                cmax = wp.tile([P, 1], F32, tag="cmax")
                nc.vector.tensor_scalar_max(out=cmax[:], in0=prt[:, DOUT:DOUT + 1],
                                            scalar1=1.0)
                rec = wp.tile([P, 1], F32, tag="rec")
                nc.vector.reciprocal(out=rec[:], in_=cmax[:])
                mean = wp.tile([P, DOUT], F32, tag="mean")
                nc.vector.tensor_tensor(out=mean[:], in0=rec[:].to_broadcast([P, DOUT]),
                                        in1=prt[:, 0:DOUT], op=MULT)
                osb = wp.tile([P, DOUT], F32, tag="osb")
                nc.vector.tensor_tensor(out=osb[:], in0=mean[:], in1=c_bc[:], op=ADD)
                nc.sync.dma_start(out=out_t[gt * P:(gt + 1) * P, :], in_=osb[:])

    nc.compile()
    return nc


def _host_prep(inputs):
    """Shard + label edges; build per-core input maps. Index work only."""
    x = np.asarray(inputs["x"])
    edge_index = np.asarray(inputs["edge_index"])
    batch = np.asarray(inputs["batch"]).astype(np.int64)
    N = x.shape[0]
    NPC = math.ceil(N / NCORE)        # nodes per core
    NT = math.ceil(NPC / P)           # node tiles per core
    NPAD = NT * P

    src = edge_index[0].astype(np.int64)
    dst = edge_index[1].astype(np.int64)
    deg = np.bincount(dst, minlength=N)
    maxdeg = int(deg.max())
    C = 32
    while C <= maxdeg:
        C *= 2
    assert C <= 128

    # append self-loop edges: the GIN own term h1[i] has class deg[i]
    allsrc = np.concatenate([src, np.arange(N, dtype=np.int64)])
    alldst = np.concatenate([dst, np.arange(N, dtype=np.int64)])
    E = alldst.shape[0]

    order = np.argsort(alldst, kind="stable")
    dsts = alldst[order]
    clss = deg[allsrc[order]]

    core = dsts // NPC
    tile_idx = (dsts % NPC) // P
    kt = core * NT + tile_idx
    cnt_kt = np.bincount(kt, minlength=NCORE * NT)
    ct = int(math.ceil(cnt_kt.max() / P))
    CH = NT * ct

    starts = np.zeros(NCORE * NT, np.int64)
    starts[1:] = np.cumsum(cnt_kt)[:-1]
    rank = np.arange(E) - starts[kt]
    col = tile_idx * ct + rank // P
    row = rank % P

    dstrel = np.full((NCORE, P, CH), SENT, np.float32)
    clssrc = np.full((NCORE, P, CH), SENT, np.float32)
    dstrel[core, row, col] = (dsts % NPC) % P
    clssrc[core, row, col] = clss

    # graph labels, padded per core
    batchpad = np.full((NCORE, NPAD), -1, np.int64)
    for k in range(NCORE):
        lo, hi = k * NPC, min((k + 1) * NPC, N)
        batchpad[k, 0:hi - lo] = batch[lo:hi]

    B0 = np.array([batch[min(k * NPC, N - 1)] // P for k in range(NCORE)])
    spread = np.array([
        (batch[min((k + 1) * NPC, N) - 1] // P) - B0[k] for k in range(NCORE)])
    W = int(P * (spread.max() + 1))
    W = max(W, 2 * P)
    assert W <= 256, "graph window exceeds bf16-exact range; need f32 fallback"
    brel = batchpad - (B0 * P)[:, None].astype(np.int64)
    brel = np.where(batchpad < 0, int(SENT), brel).astype(np.float32)
    batchrel = brel.reshape(NCORE, NT, P).transpose(0, 2, 1)

    pmat = np.zeros((NCORE, W, GRAPHS), np.float32)
    for k in range(NCORE):
        rows = np.arange(W)
        gcols = B0[k] * P + rows
        valid = gcols < GRAPHS
        pmat[k, rows[valid], gcols[valid]] = 1.0

    consts = {
        "iota_s": np.tile(np.arange(P, dtype=np.float32), ct)[None, :].repeat(P, 0),
        "iota_c": np.tile(np.arange(C, dtype=np.float32), ct)[None, :].repeat(P, 0),
        "iota_g": np.arange(W, dtype=np.float32)[None, :].repeat(P, 0),
        "ident_f": np.eye(P, dtype=np.float32),
        "ones_row": np.ones((1, P), np.float32),
        "dplus1": (1.0 + np.arange(C, dtype=np.float32))[None, :].repeat(P, 0),
    }
    bf16 = ml_dtypes.bfloat16
    in_maps = []
    for k in range(NCORE):
        m = {
            "dstrel": dstrel[k].astype(bf16),
            "clssrc": clssrc[k].astype(bf16),
            "batchrel": batchrel[k].astype(bf16),
            "pmat": pmat[k],
            "iota_s": consts["iota_s"].astype(bf16),
            "iota_c": consts["iota_c"].astype(bf16),
            "iota_g": consts["iota_g"].astype(bf16),
            "ident_f": consts["ident_f"],
            "ones_row": consts["ones_row"],
            "dplus1": consts["dplus1"],
            "emb": np.asarray(inputs["emb"], np.float32),
            "w1a": np.asarray(inputs["w1a"], np.float32),
            "b1a": np.asarray(inputs["b1a"], np.float32),
            "w1b": np.asarray(inputs["w1b"], np.float32),
            "b1b": np.asarray(inputs["b1b"], np.float32),
            "w2a": np.asarray(inputs["w2a"], np.float32),
            "b2a": np.asarray(inputs["b2a"], np.float32),
            "w2b": np.asarray(inputs["w2b"], np.float32),
            "b2b": np.asarray(inputs["b2b"], np.float32),
            "wf": np.asarray(inputs["wf"], np.float32),
            "bf": np.asarray(inputs["bf"], np.float32),
        }
        in_maps.append(m)
    return in_maps, C, NT, ct, W


def kernel(_trace=False, **inputs) -> np.ndarray:
    in_maps, C, NT, ct, W = _host_prep(inputs)
    nc = _build_program(C, NT, ct, W)
    res = bass_utils.run_bass_kernel_spmd(
        nc, in_maps, core_ids=list(range(NCORE)), trace=_trace)
    out = res.results[0]["out"]
    if _trace:
        kernel.last_result = res
    return out


# revision 13
# speedup vs baseline: 1.1672x; 1.1672x over previous
"""Trainium2 Bass kernel for DefaultGIN (gnn_message_passing), 8-core SPMD.

Math: with a 1-row embedding table and x==0, every node's input feature is
emb[0], so conv1's output depends only on each node's in-degree d:
    h1[i] = relu(mlp1((1+deg_i) * e))          (table F[d], d < C classes)
Conv2's pre-activation collapses to a per-node class-count matmul:
    pre2a[i] = (h1[i] + sum_j h1[j]) @ w2a + b2a
             = CNT[i] @ G'            where G'[d] = F[d] @ w2a, plus a bias row,
    CNT[i,d] = #{in-neighbors j: deg_j = d} + 1[deg_i = d]
(the own term enters as a host-added self-loop edge), and the trailing
linear layers commute with mean pooling:
    out = (sum_g relu(pre2a) @ (w2b @ wf)) / count_g + (b2b @ wf + bf).

Device does all float math: tables, per-edge scatter-accumulation of CNT
(one-hot compare + PE matmul per 128-edge chunk; counts are exact in bf16),
pre2a via a hi/lo bf16 split of G' (near-fp32 precision at bf16 speed),
relu, pooling, tail matmuls, cross-core AllReduce, and the mean division.
Host only shards and labels edges (sort by dst, degree-class ints) and
builds iota/identity constants.
"""

import math

import numpy as np
import ml_dtypes

import concourse.bass as bass
import concourse.bacc as bacc
import concourse.mybir as mybir
import concourse.tile as tile
from concourse import bass_utils

P = 128
NCORE = 8
D = 256
D2 = D + 2          # pooled row: 256 features + count + zero pad (even for PE)
DOUT = 32
GRAPHS = 512
F32 = mybir.dt.float32
BF16 = mybir.dt.bfloat16
SENT = 9999.0  # one-hot sentinel; != any iota value in bf16/f32


def _build_program(C, NT, ct, W):
    """Build the 8-core SPMD Bass program. Shapes only depend on C/NT/ct/W."""
    CH = NT * ct
    NW = W // P  # pooling window 128-blocks
    nc = bacc.Bacc("TRN2", target_bir_lowering=False)

    def din(name, shape, dt):
        return nc.dram_tensor(name, shape, dt, kind="ExternalInput")

    # per-core edge/node labels
    dstrel_t = din("dstrel", [P, CH], BF16)
    clssrc_t = din("clssrc", [P, CH], BF16)
    batchrel_t = din("batchrel", [P, NT], BF16)
    pmat_t = din("pmat", [W, GRAPHS], F32)
    # constants
    iota_s_t = din("iota_s", [P, ct * P], BF16)
    iota_c_t = din("iota_c", [P, ct * C], BF16)
    iota_g_t = din("iota_g", [P, W], BF16)
    ident_f_t = din("ident_f", [P, P], F32)
    ones_row_t = din("ones_row", [1, P], F32)
    dplus1_t = din("dplus1", [P, C], F32)
    # weights
    emb_t = din("emb", [1, D], F32)
    w1a_t = din("w1a", [D, D], F32)
    b1a_t = din("b1a", [D], F32)
    w1b_t = din("w1b", [D, D], F32)
    b1b_t = din("b1b", [D], F32)
    w2a_t = din("w2a", [D, D], F32)
    b2a_t = din("b2a", [D], F32)
    w2b_t = din("w2b", [D, D], F32)
    b2b_t = din("b2b", [D], F32)
    wf_t = din("wf", [D, DOUT], F32)
    bf_t = din("bf", [DOUT], F32)

    out_t = nc.dram_tensor("out", [GRAPHS, DOUT], F32, kind="ExternalOutput")

    pg_t = nc.dram_tensor("pg", [GRAPHS, DOUT + 2], F32)
    pr_t = nc.dram_tensor("pr", [NCORE * GRAPHS, DOUT + 2], F32,
                          addr_space="Shared")

    IS_EQ = mybir.AluOpType.is_equal
    MULT = mybir.AluOpType.mult
    ADD = mybir.AluOpType.add
    SUB = mybir.AluOpType.subtract
    RELU = mybir.ActivationFunctionType.Relu

    with tile.TileContext(nc) as tc:
        with (
            tc.tile_pool(name="const", bufs=1) as cp,
            tc.tile_pool(name="work", bufs=4) as wp,
            tc.tile_pool(name="psum", bufs=5, space="PSUM") as pp,
            tc.tile_pool(name="pwin", bufs=1, space="PSUM") as pwp,
        ):
            # ---- load constants & weights ----
            def load(pool, t):
                tl = pool.tile(list(t.shape), t.dtype, tag=t.name)
                nc.sync.dma_start(out=tl[:], in_=t[:])
                return tl

            dstrel = load(cp, dstrel_t)
            clssrc = load(cp, clssrc_t)
            batchrel = load(cp, batchrel_t)
            iota_s = load(cp, iota_s_t)
            iota_c = load(cp, iota_c_t)
            iota_g = load(cp, iota_g_t)
            ident_f = load(cp, ident_f_t)
            ones_row = load(cp, ones_row_t)
            dplus1 = load(cp, dplus1_t)

            def load_chunks(t, n, cols):
                out = []
                for kc in range(n):
                    tl = cp.tile([P, cols], F32, tag=f"{t.name}_ch{kc}")
                    nc.sync.dma_start(out=tl[:], in_=t[kc * P:(kc + 1) * P, :])
                    out.append(tl)
                return out

            w1a_sb = load_chunks(w1a_t, 2, D)
            w1b_sb = load_chunks(w1b_t, 2, D)
            w2a_sb = load_chunks(w2a_t, 2, D)
            w2b_sb = load_chunks(w2b_t, 2, D)
            wf_sb = load_chunks(wf_t, 2, DOUT)
            pmat_sb = load_chunks(pmat_t, NW, GRAPHS)

            def load_row(t, n):
                tl = cp.tile([1, n], F32, tag=f"{t.name}_row")
                nc.sync.dma_start(out=tl[:], in_=t[None, :])
                return tl

            b1a_row = load_row(b1a_t, D)
            b1b_row = load_row(b1b_t, D)
            b2a_row = load_row(b2a_t, D)
            bf_row = load_row(bf_t, DOUT)
            b2b_col = []
            for kc in range(2):
                tl = cp.tile([P, 1], F32, tag=f"b2bc{kc}")
                nc.sync.dma_start(out=tl[:], in_=b2b_t[kc * P:(kc + 1) * P, None])
                b2b_col.append(tl)
            e_chunk = []
            for kc in range(2):
                tl = cp.tile([P, 1], F32, tag=f"ech{kc}")
                nc.sync.dma_start(out=tl[:], in_=emb_t[0, kc * P:(kc + 1) * P, None])
                e_chunk.append(tl)

            # ---- P0: degree-class tables F, G' and folded tail weights ----
            def mlp_layer(lhsT_chunks, w_chunks, b_row, relu_out_tile):
                ps = pp.tile([P, D2], F32, space="PSUM", tag="ps")
                for kc in range(2):
                    nc.tensor.matmul(out=ps[0:C, 0:D], lhsT=lhsT_chunks[kc][:],
                                     rhs=w_chunks[kc][:], start=(kc == 0), stop=False)
                nc.tensor.matmul(out=ps[0:C, 0:D], lhsT=ones_row[:, 0:C], rhs=b_row[:],
                                 start=False, stop=True)
                nc.scalar.activation(out=relu_out_tile[:], in_=ps[0:C, 0:D], func=RELU)

            def transpose_cols(src_tile, np_, tags):
                # src [np_, 256] f32 -> two sbuf tiles [128, np_] (k-chunks)
                out = []
                for kc in range(2):
                    ps = pp.tile([P, D2], F32, space="PSUM", tag="ps")
                    nc.tensor.transpose(out=ps[0:P, 0:np_],
                                        in_=src_tile[:, kc * P:(kc + 1) * P],
                                        identity=ident_f[0:np_, 0:np_])
                    tl = wp.tile([P, np_], F32, tag=f"{tags}{kc}")
                    nc.vector.tensor_copy(out=tl[:], in_=ps[0:P, 0:np_])
                    out.append(tl)
                return out

            z0T = []
            for kc in range(2):
                tl = wp.tile([P, C], F32, tag=f"z0T{kc}")
                nc.vector.tensor_tensor(out=tl[:], in0=e_chunk[kc][:].to_broadcast([P, C]),
                                        in1=dplus1[:], op=MULT)
                z0T.append(tl)
            h1 = cp.tile([C, D], F32, tag="h1")
            mlp_layer(z0T, w1a_sb, b1a_row, h1)
            h1T = transpose_cols(h1, C, "h1T")
            fv = cp.tile([C, D], F32, tag="fv")
            mlp_layer(h1T, w1b_sb, b1b_row, fv)
            fvT = transpose_cols(fv, C, "fvT")
            gps = pp.tile([P, D2], F32, space="PSUM", tag="ps")
            for kc in range(2):
                nc.tensor.matmul(out=gps[0:C, 0:D], lhsT=fvT[kc][:], rhs=w2a_sb[kc][:],
                                 start=(kc == 0), stop=(kc == 1))
            # G'' staging (f32): rows 0..C-1 = F@w2a, row C = b2a; col 256 marks
            # the bias row (always-1 count -> relu(1)=1 feeds pooled counts),
            # col 257 = 0.
            gp_f = cp.tile([C + 1, D2], F32, tag="gp_f")
            nc.vector.tensor_copy(out=gp_f[0:C, 0:D], in_=gps[0:C, 0:D])
            nc.vector.tensor_copy(out=gp_f[C:C + 1, 0:D], in_=b2a_row[:])
            nc.gpsimd.memset(gp_f[0:C + 1, D:D2], 0.0)
            nc.vector.tensor_copy(out=gp_f[C:C + 1, D:D + 1], in_=ones_row[:, 0:1])
            # hi/lo bf16 split: gp_hi + gp_lo ~= gp_f to ~16 mantissa bits
            gp_hi = cp.tile([C + 1, D2], BF16, tag="gp_hi")
            nc.vector.tensor_copy(out=gp_hi[:], in_=gp_f[:])
            gp_lo_f = cp.tile([C + 1, D2], F32, tag="gp_lo_f")
            nc.vector.tensor_tensor(out=gp_lo_f[:], in0=gp_f[:], in1=gp_hi[:], op=SUB)
            gp_lo = cp.tile([C + 1, D2], BF16, tag="gp_lo")
            nc.vector.tensor_copy(out=gp_lo[:], in_=gp_lo_f[:])

            # W2 = w2b @ wf  (needs w2b blocks transposed), c = b2b @ wf + bf
            w2sb = []
            for ic in range(2):
                ps = pp.tile([P, D2], F32, space="PSUM", tag="ps")
                for kc in range(2):
                    tps = pp.tile([P, D2], F32, space="PSUM", tag="ps")
                    nc.tensor.transpose(out=tps[0:P, 0:P],
                                        in_=w2b_sb[ic][:, kc * P:(kc + 1) * P],
                                        identity=ident_f[:])
                    tsb = wp.tile([P, P], F32, tag="w2bT")
                    nc.vector.tensor_copy(out=tsb[:], in_=tps[0:P, 0:P])
                    nc.tensor.matmul(out=ps[0:P, 0:DOUT], lhsT=tsb[:], rhs=wf_sb[kc][:],
                                     start=(kc == 0), stop=(kc == 1))
                tl = cp.tile([P, DOUT], F32, tag=f"w2sb{ic}")
                nc.vector.tensor_copy(out=tl[:], in_=ps[0:P, 0:DOUT])
                w2sb.append(tl)
            cps = pp.tile([P, D2], F32, space="PSUM", tag="ps")
            for kc in range(2):
                nc.tensor.matmul(out=cps[0:1, 0:DOUT], lhsT=b2b_col[kc][:],
                                 rhs=wf_sb[kc][:], start=(kc == 0), stop=False)
            nc.tensor.matmul(out=cps[0:1, 0:DOUT], lhsT=ones_row[:, 0:1], rhs=bf_row[:],
                             start=False, stop=True)
            c_row = cp.tile([1, DOUT], F32, tag="c_row")
            nc.vector.tensor_copy(out=c_row[:], in_=cps[0:1, 0:DOUT])
            cbc_ps = pp.tile([P, D2], F32, space="PSUM", tag="ps")
            nc.tensor.matmul(out=cbc_ps[0:P, 0:DOUT], lhsT=ones_row[:], rhs=c_row[:],
                             start=True, stop=True)
            c_bc = cp.tile([P, DOUT], F32, tag="c_bc")
            nc.vector.tensor_copy(out=c_bc[:], in_=cbc_ps[0:P, 0:DOUT])

            # ---- P1: per-tile edge aggregation, pre2a, relu, pooling ----
            pw_ps = [pwp.tile([P, D2], F32, space="PSUM", tag=f"pw{h}",
                              name=f"pw{h}")
                     for h in range(NW)]
            for t in range(NT):
                s_all = wp.tile([P, ct * P], BF16, tag="s_all")
                nc.vector.tensor_tensor(
                    out=s_all[:],
                    in0=dstrel[:, t * ct:(t + 1) * ct].to_broadcast([P, ct, P]),
                    in1=iota_s[:], op=IS_EQ)
                ohc = wp.tile([P, ct * C], BF16, tag="ohc")
                nc.vector.tensor_tensor(
                    out=ohc[:],
                    in0=clssrc[:, t * ct:(t + 1) * ct].to_broadcast([P, ct, C]),
                    in1=iota_c[:], op=IS_EQ)

                ctps = pp.tile([P, D2], F32, space="PSUM", tag="ps")
                for c in range(ct):
                    nc.tensor.matmul(out=ctps[0:C, 0:P], lhsT=ohc[:, c * C:(c + 1) * C],
                                     rhs=s_all[:, c * P:(c + 1) * P],
                                     start=(c == 0), stop=(c == ct - 1))

                ctsb = wp.tile([C + 1, P], BF16, tag="ctsb")
                nc.vector.tensor_copy(out=ctsb[0:C, :], in_=ctps[0:C, 0:P])
                nc.vector.tensor_copy(out=ctsb[C:C + 1, :], in_=ones_row[:])

                preps = pp.tile([P, D2], F32, space="PSUM", tag="ps")
                nc.tensor.matmul(out=preps[:], lhsT=ctsb[:], rhs=gp_hi[:],
                                 start=True, stop=False)
                nc.tensor.matmul(out=preps[:], lhsT=ctsb[:], rhs=gp_lo[:],
                                 start=False, stop=True)
                msb = wp.tile([P, D2], BF16, tag="msb")
                nc.scalar.activation(out=msb[:], in_=preps[:], func=RELU)

                ohg = wp.tile([P, W], BF16, tag="ohg")
                nc.vector.tensor_tensor(
                    out=ohg[:], in0=batchrel[:, t:t + 1].to_broadcast([P, W]),
                    in1=iota_g[:], op=IS_EQ)
                for h in range(NW):
                    nc.tensor.matmul(out=pw_ps[h][:], lhsT=ohg[:, h * P:(h + 1) * P],
                                     rhs=msb[:], start=(t == 0), stop=(t == NT - 1),
                                     skip_group_check=True)

            # ---- P2: local tail matmul, placement, small AllReduce, mean ----
            pw_sb = []
            for h in range(NW):
                tl = wp.tile([P, D2], F32, tag=f"pwsb{h}")
                nc.vector.tensor_copy(out=tl[:], in_=pw_ps[h][:])
                pw_sb.append(tl)
            # per window block h: pre_h = pooled_h @ W2 (divide later), + counts
            pre_h_sb = []
            for h in range(NW):
                ops = pp.tile([P, D2], F32, space="PSUM", tag="ps")
                for fc in range(2):
                    tps = pp.tile([P, D2], F32, space="PSUM", tag="ps")
                    nc.tensor.transpose(out=tps[0:P, 0:P],
                                        in_=pw_sb[h][:, fc * P:(fc + 1) * P],
                                        identity=ident_f[:])
                    pwT = wp.tile([P, P], F32, tag="pwT")
                    nc.vector.tensor_copy(out=pwT[:], in_=tps[0:P, 0:P])
                    nc.tensor.matmul(out=ops[0:P, 0:DOUT], lhsT=pwT[:],
                                     rhs=w2sb[fc][:], start=(fc == 0), stop=(fc == 1))
                tl = wp.tile([P, DOUT + 2], F32, tag=f"preh{h}")
                nc.vector.tensor_copy(out=tl[:, 0:DOUT], in_=ops[0:P, 0:DOUT])
                nc.vector.tensor_copy(out=tl[:, DOUT:DOUT + 1],
                                      in_=pw_sb[h][:, D:D + 1])
                nc.vector.tensor_copy(out=tl[:, DOUT + 1:DOUT + 2],
                                      in_=pw_sb[h][:, D + 1:D + 2])
                pre_h_sb.append(tl)
            write_dmas = []
            for gt in range(GRAPHS // P):
                ps = pp.tile([P, D2], F32, space="PSUM", tag="ps")
                for h in range(NW):
                    nc.tensor.matmul(out=ps[0:P, 0:DOUT + 2],
                                     lhsT=pmat_sb[h][:, gt * P:(gt + 1) * P],
                                     rhs=pre_h_sb[h][:], start=(h == 0),
                                     stop=(h == NW - 1))
                tl = wp.tile([P, DOUT + 2], F32, tag="plsb")
                nc.vector.tensor_copy(out=tl[:], in_=ps[0:P, 0:DOUT + 2])
                dma = nc.sync.dma_start(out=pg_t[gt * P:(gt + 1) * P, :], in_=tl[:])
                write_dmas.append(dma)

            cc = nc.gpsimd.collective_compute(
                "AllGather", mybir.AluOpType.bypass,
                replica_groups=[list(range(NCORE))],
                ins=[pg_t[:]], outs=[pr_t[:]])
            for dma in write_dmas:
                tile.add_dep_helper(cc.ins, dma.ins, reason="allreduce after writes")

            prv = pr_t.reshape([NCORE, GRAPHS, DOUT + 2])
            DO2 = DOUT + 2
            for gt in range(GRAPHS // P):
                agt = wp.tile([P, NCORE * DO2], F32, tag="agt")
                dma = nc.sync.dma_start(
                    out=agt[:],
                    in_=prv[:, gt * P:(gt + 1) * P, :].transpose([1, 0, 2]))
                tile.add_dep_helper(dma.ins, cc.ins, reason="read after allgather")
                s4 = wp.tile([P, 4 * DO2], F32, tag="s4")
                nc.vector.tensor_tensor(out=s4[:], in0=agt[:, 0:4 * DO2],
                                        in1=agt[:, 4 * DO2:8 * DO2], op=ADD)
                s2 = wp.tile([P, 2 * DO2], F32, tag="s2")
                nc.vector.tensor_tensor(out=s2[:], in0=s4[:, 0:2 * DO2],
                                        in1=s4[:, 2 * DO2:4 * DO2], op=ADD)
                prt = wp.tile([P, DO2], F32, tag="prt")
                nc.vector.tensor_tensor(out=prt[:], in0=s2[:, 0:DO2],
                                        in1=s2[:, DO2:2 * DO2], op=ADD)
                cmax = wp.tile([P, 1], F32, tag="cmax")
                nc.vector.tensor_scalar_max(out=cmax[:], in0=prt[:, DOUT:DOUT + 1],
                                            scalar1=1.0)
                rec = wp.tile([P, 1], F32, tag="rec")
                nc.vector.reciprocal(out=rec[:], in_=cmax[:])
                mean = wp.tile([P, DOUT], F32, tag="mean")
                nc.vector.tensor_tensor(out=mean[:], in0=rec[:].to_broadcast([P, DOUT]),
                                        in1=prt[:, 0:DOUT], op=MULT)
                osb = wp.tile([P, DOUT], F32, tag="osb")
                nc.vector.tensor_tensor(out=osb[:], in0=mean[:], in1=c_bc[:], op=ADD)
                nc.sync.dma_start(out=out_t[gt * P:(gt + 1) * P, :], in_=osb[:])

    nc.compile()
    return nc


def _host_prep(inputs):
    """Shard + label edges; build per-core input maps. Index work only."""
    x = np.asarray(inputs["x"])
    edge_index = np.asarray(inputs["edge_index"])
    batch = np.asarray(inputs["batch"]).astype(np.int64)
    N = x.shape[0]
    NPC = math.ceil(N / NCORE)        # nodes per core

    src = edge_index[0].astype(np.int64)
    dst = edge_index[1].astype(np.int64)
    deg = np.bincount(dst, minlength=N)
    maxdeg = int(deg.max())
    C = 32
    while C <= maxdeg:
        C *= 2
    assert C <= 128

    # Balance edge load across node tiles: serpentine-deal nodes (by
    # in-degree+1, desc) into NT bins per core, so every tile has ~equal
    # edge count and the uniform chunk count ct stays minimal.
    def pack(NT):
        newpos = np.empty(N, np.int64)
        loads = np.zeros((NCORE, NT), np.int64)
        for k in range(NCORE):
            lo, hi = k * NPC, min((k + 1) * NPC, N)
            nodes = np.arange(lo, hi)
            sizes = deg[lo:hi] + 1
            order_ = np.argsort(-sizes, kind="stable")
            nn = hi - lo
            rows = (np.arange(nn)) // NT
            cols = (np.arange(nn)) % NT
            cols = np.where(rows % 2 == 1, NT - 1 - cols, cols)  # serpentine
            # position within bin = row index; bins may have uneven fill at end
            binfill = np.zeros(NT, np.int64)
            # vectorized: slot = row (since each row fills each bin at most once)
            slot = rows
            newpos[nodes[order_]] = k * 0 + cols * P + slot
            np.add.at(loads[k], cols, sizes[order_])
            assert slot.max() < P
        return newpos, int(loads.max())

    NT = math.ceil(NPC / P)
    newpos, maxload = pack(NT)
    ct = math.ceil(maxload / P)
    if NT * ct > (NT + 1) * math.ceil(maxload / P - 0.5):  # try one more tile
        newpos2, maxload2 = pack(NT + 1)
        if (NT + 1) * math.ceil(maxload2 / P) < NT * ct:
            NT, newpos, maxload = NT + 1, newpos2, maxload2
            ct = math.ceil(maxload / P)
    NPAD = NT * P
    CH = NT * ct

    # append self-loop edges: the GIN own term h1[i] has class deg[i]
    allsrc = np.concatenate([src, np.arange(N, dtype=np.int64)])
    alldst = np.concatenate([dst, np.arange(N, dtype=np.int64)])
    E = alldst.shape[0]

    dcore = alldst // NPC
    dpos = newpos[alldst]
    order = np.argsort(dcore * NPAD + dpos, kind="stable")
    dsts = alldst[order]
    clss = deg[allsrc[order]]
    core = dcore[order]
    pos = dpos[order]
    tile_idx = pos // P

    kt = core * NT + tile_idx
    cnt_kt = np.bincount(kt, minlength=NCORE * NT)
    assert int(math.ceil(cnt_kt.max() / P)) <= ct

    starts = np.zeros(NCORE * NT, np.int64)
    starts[1:] = np.cumsum(cnt_kt)[:-1]
    rank = np.arange(E) - starts[kt]
    col = tile_idx * ct + rank // P
    row = rank % P

    dstrel = np.full((NCORE, P, CH), SENT, np.float32)
    clssrc = np.full((NCORE, P, CH), SENT, np.float32)
    dstrel[core, row, col] = pos % P
    clssrc[core, row, col] = clss

    # graph labels at permuted positions, padded per core
    batchpad = np.full((NCORE, NPAD), -1, np.int64)
    gbase = np.zeros(NCORE, np.int64)
    for k in range(NCORE):
        lo, hi = k * NPC, min((k + 1) * NPC, N)
        batchpad[k, newpos[lo:hi]] = batch[lo:hi]
        gbase[k] = batch[lo]
        assert batch[hi - 1] - batch[lo] < P - 1

    W = P
    brel = batchpad - gbase[:, None]
    brel = np.where(batchpad < 0, int(SENT), brel).astype(np.float32)
    batchrel = brel.reshape(NCORE, NT, P).transpose(0, 2, 1)

    pmat = np.zeros((NCORE, W, GRAPHS), np.float32)
    for k in range(NCORE):
        rows = np.arange(W)
        gcols = gbase[k] + rows
        valid = gcols < GRAPHS
        pmat[k, rows[valid], gcols[valid]] = 1.0

    consts = {
        "iota_s": np.tile(np.arange(P, dtype=np.float32), ct)[None, :].repeat(P, 0),
        "iota_c": np.tile(np.arange(C, dtype=np.float32), ct)[None, :].repeat(P, 0),
        "iota_g": np.arange(W, dtype=np.float32)[None, :].repeat(P, 0),
        "ident_f": np.eye(P, dtype=np.float32),
        "ones_row": np.ones((1, P), np.float32),
        "dplus1": (1.0 + np.arange(C, dtype=np.float32))[None, :].repeat(P, 0),
    }
    bf16 = ml_dtypes.bfloat16
    in_maps = []
    for k in range(NCORE):
        m = {
            "dstrel": dstrel[k].astype(bf16),
            "clssrc": clssrc[k].astype(bf16),
            "batchrel": batchrel[k].astype(bf16),
            "pmat": pmat[k],
            "iota_s": consts["iota_s"].astype(bf16),
            "iota_c": consts["iota_c"].astype(bf16),
            "iota_g": consts["iota_g"].astype(bf16),
            "ident_f": consts["ident_f"],
            "ones_row": consts["ones_row"],
            "dplus1": consts["dplus1"],
            "emb": np.asarray(inputs["emb"], np.float32),
            "w1a": np.asarray(inputs["w1a"], np.float32),
            "b1a": np.asarray(inputs["b1a"], np.float32),
            "w1b": np.asarray(inputs["w1b"], np.float32),
            "b1b": np.asarray(inputs["b1b"], np.float32),
            "w2a": np.asarray(inputs["w2a"], np.float32),
            "b2a": np.asarray(inputs["b2a"], np.float32),
            "w2b": np.asarray(inputs["w2b"], np.float32),
            "b2b": np.asarray(inputs["b2b"], np.float32),
            "wf": np.asarray(inputs["wf"], np.float32),
            "bf": np.asarray(inputs["bf"], np.float32),
        }
        in_maps.append(m)
    return in_maps, C, NT, ct, W


def kernel(_trace=False, **inputs) -> np.ndarray:
    in_maps, C, NT, ct, W = _host_prep(inputs)
    nc = _build_program(C, NT, ct, W)
    res = bass_utils.run_bass_kernel_spmd(
        nc, in_maps, core_ids=list(range(NCORE)), trace=_trace)
    out = res.results[0]["out"]
    if _trace:
        kernel.last_result = res
    return out
